# revision 26
# baseline (speedup 1.0000x reference)
"""TRN2 Bass kernel for nn_CrossAttentionHeightSplit (v2).

26-view cross-attention, 2 scenes, C=256, 8 heads x d=32, q=1024 tokens/view,
kv = 3-4 neighbor views (1024 tokens each), 5 shared weight groups.

Design (per-core SPMD over 8 cores):
  Sharding: each core gets 4 full 4-neighbor views + 2 full 3-neighbor views
  + 1 half (512-q) 4-neighbor view => identical static program, balanced
  score/AV work (sum n = 24 kv-views per core + 2 extra for the half).

  The bottleneck is softmax-exp evacuation of scores from PSUM (201M
  elements/core). Both evac engines run in parallel, statically
  load-balanced:
    - ACT: native exp psum fp32 -> sbuf bf16 (1 elem/cycle/lane)
    - DVE: Schraudolph exp-bit-trick: tensor_scalar(mult,add) psum fp32 ->
      int16 (= bf16 bits of exp(x*ISQ)), 1 elem/cycle/lane
  Score matmuls (K=32) use 4-way row tile_position concurrency; AV matmuls
  (33-wide stationary with the ones-column softmax-denominator trick) use
  2-way column tile_position concurrency. Projections are bf16 (x and W
  pre-converted on host, halving DMA).

  PSUM budget (8 banks): 3 x [128,1024] rotating (scores + projections +
  out-proj) + 1 x [97,1024] AV accumulator (both head-pairs of a quad).
  Software pipelining: AV for chunk c is emitted after scores for chunk
  c+2; normalization/out-proj of a quad is deferred into the next quad's
  chunk loop to avoid head-of-line stalls on the engine FIFOs.
"""

import sys
import numpy as np

try:
    import concourse.bass as bass  # noqa: F401
except ImportError:
    sys.path.insert(0, "/opt/trn_rl_repo")

import ml_dtypes
import concourse.bacc as bacc
import concourse.mybir as mybir
import concourse.tile as tile
from concourse.bass_utils import run_bass_kernel_spmd

dt = mybir.dt
AF = mybir.ActivationFunctionType

# ---------------------------------------------------------------- constants
N_VIEWS = 26
C = 256
S = 1024
NH = 8
D = 32
ISQ = float(1.0 / np.sqrt(D))

# Schraudolph exp for bf16 target: bits(exp(x*ISQ)) ~= x*SCH_A + SCH_B
SCH_A = ISQ * 128.0 / float(np.log(2.0))
SCH_B = 16256.0 - 7.45          # adjusted after probe (rounding mode)

SEL = {
    0: [18, 20, 22, 24], 1: [2, 4, 6, 8], 2: [1, 3, 9, 10], 3: [2, 4, 11],
    4: [1, 3, 5, 12], 5: [4, 6, 13], 6: [1, 5, 7, 14], 7: [6, 8, 15],
    8: [1, 7, 9, 16], 9: [2, 8, 17], 10: [2, 11, 17, 18], 11: [3, 10, 12, 19],
    12: [4, 11, 13, 20], 13: [5, 12, 14, 21], 14: [6, 13, 15, 22],
    15: [7, 14, 16, 23], 16: [8, 15, 17, 24], 17: [9, 10, 16, 25],
    18: [0, 10, 19, 25], 19: [11, 18, 20], 20: [0, 12, 19, 21],
    21: [13, 20, 22], 22: [0, 14, 21, 23], 23: [15, 22, 24],
    24: [0, 16, 23, 25], 25: [17, 18, 24],
}
MHA_IDX = [0, 1] + [2] * 8 + [3] * 8 + [4] * 8

N_CORES = 8
SLOT_N = [4, 4, 4, 4, 3, 3, 4]       # neighbors per slot
SLOT_Q = [1024] * 6 + [512]          # q tokens per slot (slot 6 = half view)
N_SLOTS = 7
KVOFF = [0, 4, 8, 12, 16, 19, 22]
KV_ROWS = 26

_V4 = [i for i in range(N_VIEWS) if len(SEL[i]) == 4]   # 18 views
_V3 = [i for i in range(N_VIEWS) if len(SEL[i]) == 3]   # 8 views
_T4 = [(b, i) for b in range(2) for i in _V4]           # 36
_T3 = [(b, i) for b in range(2) for i in _V3]           # 16

_PROGRAM_CACHE = {}
DEBUG = False          # adds intermediate dumps for slot 0 / qh 0 / mq 0


def _core_slots(core):
    """Per-slot (b, view, qhalf_or_None) for one core."""
    slots = []
    for k in range(4):
        slots.append((*_T4[4 * core + k], None))
    for k in range(2):
        slots.append((*_T3[2 * core + k], None))
    b, i = _T4[32 + core // 2]
    slots.append((b, i, core % 2))
    return slots


class _Evac:
    """Greedy static load balancer for PSUM->SBUF evacuation ops."""

    def __init__(self, nc):
        self.nc = nc
        self.t_act = 0.0
        self.t_dve = 0.0

    def _pick(self, fd):
        ca = (fd + 311.0) / 1.2     # measured: 1112ns @ fd=1024
        cd = (fd + 149.0) / 0.96    # measured: 1222ns @ fd=1024
        if self.t_act + ca <= self.t_dve + cd:
            self.t_act += ca
            return "act"
        self.t_dve += cd
        return "dve"

    def dve_cost(self, ns):
        self.t_dve += ns

    def exp(self, pool, name, src, fd):
        """exp(src*ISQ) -> fresh bf16-readable sbuf tile [128, fd]."""
        eng = self._pick(fd)
        if eng == "act":
            t = pool.tile([128, fd], dt.bfloat16, tag="esA", name=name + "a")
            self.nc.scalar.activation(t, src, AF.Exp, scale=ISQ)
            return t
        t = pool.tile([128, fd], dt.int16, tag="esD", name=name + "d")
        self.nc.vector.tensor_scalar(t, src, SCH_A, SCH_B,
                                     mybir.AluOpType.mult, mybir.AluOpType.add)
        return t.bitcast(dt.bfloat16)

    def copy(self, dst, src, fd):
        eng = self._pick(fd)
        if eng == "act":
            self.nc.scalar.copy(dst, src)
        else:
            self.nc.vector.tensor_copy(dst, src)


def _build_program():
    if "nc" in _PROGRAM_CACHE:
        return _PROGRAM_CACHE["nc"]

    nc = bacc.Bacc("TRN2", target_bir_lowering=False, debug=False)
    f32, bf16, i16 = dt.float32, dt.bfloat16, dt.int16

    xq_d = nc.dram_tensor("xq", [N_SLOTS, C, S], bf16, kind="ExternalInput").ap()
    xkv_d = nc.dram_tensor("xkv", [KV_ROWS, C, S], bf16, kind="ExternalInput").ap()
    w_d = nc.dram_tensor("w", [N_SLOTS, C, 3 * C], bf16, kind="ExternalInput").ap()
    wo_d = nc.dram_tensor("wo", [N_SLOTS, C, C], bf16, kind="ExternalInput").ap()
    out_d = nc.dram_tensor("out", [N_SLOTS, C, S], f32, kind="ExternalOutput").ap()
    if DEBUG:
        dbg_bf_d = nc.dram_tensor("dbg_bf", [8, 128, 1024], bf16, kind="ExternalOutput").ap()
        dbg_f_d = nc.dram_tensor("dbg_f", [4, 128, 1024], f32, kind="ExternalOutput").ap()

    ev = None            # set below
    pending_early = []   # deferred normalize emissions
    pending_late = []    # deferred out-proj emissions

    from contextlib import ExitStack
    with ExitStack() as stack:
        tc = stack.enter_context(tile.TileContext(nc))
        wp = stack.enter_context(tc.tile_pool(name="wp", bufs=2))
        xqp = stack.enter_context(tc.tile_pool(name="xqp", bufs=2))
        xnp = stack.enter_context(tc.tile_pool(name="xnp", bufs=3))
        qpp = stack.enter_context(tc.tile_pool(name="qpp", bufs=2))
        kpp = stack.enter_context(tc.tile_pool(name="kpp", bufs=2))
        vpp = stack.enter_context(tc.tile_pool(name="vpp", bufs=2))
        esp = stack.enter_context(tc.tile_pool(name="esp", bufs=6))
        avp = stack.enter_context(tc.tile_pool(name="avp", bufs=4))
        rcp = stack.enter_context(tc.tile_pool(name="rcp", bufs=2))
        otp = stack.enter_context(tc.tile_pool(name="otp", bufs=2))
        psg = stack.enter_context(tc.tile_pool(name="psg", bufs=3, space="PSUM"))
        psv = stack.enter_context(tc.tile_pool(name="psv", bufs=1, space="PSUM"))

        ev = _Evac(nc)

        def flush(queue):
            for fn in queue:
                fn()
            queue.clear()

        for t in range(N_SLOTS):
            n, Q = SLOT_N[t], SLOT_Q[t]
            NQH = Q // 512

            # ---------------- projection phase ----------------
            w_sb = []
            wo_sb = []
            for ki in range(2):
                w = wp.tile([128, 3 * C], bf16, tag="w")
                nc.sync.dma_start(w, w_d[t, ki * 128:(ki + 1) * 128, :])
                w_sb.append(w)
                wo = wp.tile([128, C], bf16, tag="wo")
                nc.sync.dma_start(wo, wo_d[t, ki * 128:(ki + 1) * 128, :])
                wo_sb.append(wo)

            xq_sb = []
            for ki in range(2):
                xq = xqp.tile([128, Q], bf16, tag="xq")
                nc.sync.dma_start(xq, xq_d[t, ki * 128:(ki + 1) * 128, 0:Q])
                xq_sb.append(xq)

            # q projection -> qpT[mo] [128, Q] bf16
            qpT = []
            for mo in range(2):
                pq = psg.tile([128, 1024], f32, tag="G", name=f"pq_{t}_{mo}")
                for nq in range(NQH):
                    for ki in range(2):
                        nc.tensor.matmul(pq[:, nq * 512:(nq + 1) * 512],
                                         w_sb[ki][:, mo * 128:(mo + 1) * 128],
                                         xq_sb[ki][:, nq * 512:(nq + 1) * 512],
                                         start=(ki == 0), stop=(ki == 1))
                q_bf = qpp.tile([128, Q], bf16, tag="qpT")
                ev.copy(q_bf, pq[:, 0:Q], Q)
                qpT.append(q_bf)
            if DEBUG and t == 0:
                nc.sync.dma_start(dbg_bf_d[0], qpT[0])

            # k/v projections per neighbor
            kpT = [kpp.tile([128, n * S], bf16, tag="kpT", name=f"kpT{t}_{mo}")
                   for mo in range(2)]
            # per (kv-chunk, head): [v(32) | ones(32)] so the AV matmul also
            # replicates the softmax denominator across 32 partitions (the
            # later normalize needs per-lane copies; PE does this for free)
            v_sb = vpp.tile([128, 8 * n * 8 * 2 * D], bf16, tag="v", name=f"v{t}")
            nc.vector.memset(
                v_sb.rearrange("p (g h e) -> p g h e", h=NH, e=2 * D)[:, :, :, D:2 * D],
                1.0)

            for j in range(n):
                xn_sb = []
                for ki in range(2):
                    xn = xnp.tile([128, S], bf16, tag="xn")
                    nc.sync.dma_start(xn, xkv_d[KVOFF[t] + j, ki * 128:(ki + 1) * 128, :])
                    xn_sb.append(xn)
                for mo in range(2):
                    pk = psg.tile([128, 1024], f32, tag="G", name=f"pk_{t}_{j}_{mo}")
                    for nq in range(2):
                        for ki in range(2):
                            nc.tensor.matmul(pk[:, nq * 512:(nq + 1) * 512],
                                             w_sb[ki][:, C + mo * 128:C + (mo + 1) * 128],
                                             xn_sb[ki][:, nq * 512:(nq + 1) * 512],
                                             start=(ki == 0), stop=(ki == 1))
                    ev.copy(kpT[mo][:, j * S:(j + 1) * S], pk, 1024)
                for vh in range(2):
                    pv = psg.tile([128, 1024], f32, tag="G", name=f"pv_{t}_{j}_{vh}")
                    for si in range(4):
                        st = vh * 4 + si
                        for ki in range(2):
                            nc.tensor.matmul(pv[:, si * 256:(si + 1) * 256],
                                             xn_sb[ki][:, st * 128:(st + 1) * 128],
                                             w_sb[ki][:, 2 * C:3 * C],
                                             start=(ki == 0), stop=(ki == 1))
                    base = (j * 8 + vh * 4) * 8 * 2 * D
                    dst = v_sb[:, base:base + 4 * 8 * 2 * D].rearrange(
                        "p (si h e) -> p si h e", h=NH, e=2 * D)[:, :, :, 0:D]
                    src = pv.rearrange("p (si h d) -> p si h d", h=NH, d=D)
                    ev.copy(dst, src, 1024)

            if DEBUG and t == 0:
                nc.sync.dma_start(dbg_bf_d[1], kpT[0][:, 0:1024])
                nc.sync.dma_start(dbg_bf_d[2], v_sb[:, 0:1024])

            # ---------------- attention phase ----------------
            NCH = 8 * n
            for qh in range(NQH):
                avnT = [None, None]
                for mq in range(2):
                    pav_box = [None]
                    es_tiles = [None] * NCH

                    def emit_av(c, pav_box=pav_box, es_tiles=es_tiles, mq=mq,
                                v_sb=v_sb, NCH=NCH):
                        esb = es_tiles[c]
                        pav = pav_box[0]
                        st_, sp_ = (c == 0), (c == NCH - 1)
                        for p in range(2):          # pair index
                            for hi in range(2):     # lo/hi within pair
                                loc = 2 * p + hi
                                g = 4 * mq + loc    # global head
                                rows = pav[0:64, p * 512:(p + 1) * 512] if hi == 0 \
                                    else pav[64:128, p * 512:(p + 1) * 512]
                                cg = 0 if hi == 0 else 64
                                off = (c * 8 + g) * 2 * D
                                nc.tensor.matmul(
                                    rows, v_sb[:, off:off + 2 * D], esb[loc],
                                    start=st_, stop=sp_, tile_position=(0, cg))
                        es_tiles[c] = None

                    for c in range(NCH):
                        if c == 2:
                            # old pav's readers flush first, then take the slot
                            flush(pending_early)
                            pav_box[0] = psv.tile([128, 1024], f32, tag="pav",
                                                  name=f"pav_{t}_{qh}_{mq}")
                        if c == 6:
                            flush(pending_late)
                        # scores: 4 heads, 4-way row tiling, 2 G tiles
                        G = psg.tile([128, 1024], f32, tag="G", name=f"g_{t}_{qh}_{mq}_{c}_0")
                        G2 = psg.tile([128, 1024], f32, tag="G", name=f"g_{t}_{qh}_{mq}_{c}_1")
                        for loc in range(4):
                            tgt = G if loc < 2 else G2
                            col = (loc % 2) * 512
                            nc.tensor.matmul(
                                tgt[:, col:col + 512],
                                kpT[mq][32 * loc:32 * loc + 32, c * 128:(c + 1) * 128],
                                qpT[mq][32 * loc:32 * loc + 32, qh * 512:qh * 512 + 512],
                                start=True, stop=True, tile_position=(32 * loc, 0))
                        # evacuate both tiles (engine chosen greedily)
                        slices = []
                        for gi, gt in enumerate((G, G2)):
                            got = ev.exp(esp, f"es_{t}_{qh}_{mq}_{c}_{gi}", gt, 1024)
                            if DEBUG and t == 0 and qh == 0 and mq == 0 and c == 0:
                                nc.sync.dma_start(dbg_bf_d[3 + gi], got)
                            slices.append(got[:, 0:512])
                            slices.append(got[:, 512:1024])
                        es_tiles[c] = slices
                        if c >= 3:
                            emit_av(c - 3)
                    emit_av(NCH - 3)
                    emit_av(NCH - 2)
                    emit_av(NCH - 1)
                    pav = pav_box[0]
                    if DEBUG and t == 0 and qh == 0 and mq == 0:
                        pdump = otp.tile([128, 1024], f32, tag="oT", name="pavdump")
                        nc.vector.tensor_copy(pdump, pav)
                        nc.sync.dma_start(dbg_f_d[0], pdump)

                    # defer normalization into the next chunk loop
                    # pav rows: [avLo 0:32 | sumLo 32:64 | avHi 64:96 | sumHi 96:128]
                    def norm(pav=pav, mq=mq, t=t, qh=qh, avnT=avnT):
                        av_bf = avp.tile([128, 512], bf16, tag="avnT",
                                         name=f"avn_{t}_{qh}_{mq}")
                        srowA = rcp.tile([32, 1024], f32, tag="srow")
                        srowB = rcp.tile([32, 1024], f32, tag="srow")
                        ev.copy(srowA, pav[32:64, :], 1024)
                        ev.copy(srowB, pav[96:128, :], 1024)
                        recA = rcp.tile([32, 1024], f32, tag="rec")
                        recB = rcp.tile([32, 1024], f32, tag="rec")
                        nc.vector.reciprocal_approx_fast(recA, srowA)
                        nc.vector.reciprocal_approx_fast(recB, srowB)
                        ev.dve_cost(2200.0)
                        for loc in range(4):
                            p, hi = loc // 2, loc % 2
                            prow = pav[0:32, p * 512:(p + 1) * 512] if hi == 0 \
                                else pav[64:96, p * 512:(p + 1) * 512]
                            rrow = recA[:, p * 512:(p + 1) * 512] if hi == 0 \
                                else recB[:, p * 512:(p + 1) * 512]
                            nc.vector.tensor_mul(av_bf[32 * loc:32 * loc + 32, :],
                                                 prow, rrow)
                        ev.dve_cost(2760.0)
                        if DEBUG and t == 0 and qh == 0 and mq == 0:
                            nc.sync.dma_start(dbg_f_d[1][0:32], recA)
                            nc.sync.dma_start(dbg_f_d[2][0:32], recB)
                            nc.sync.dma_start(dbg_bf_d[5][:, 0:512], av_bf)
                        avnT[mq] = av_bf
                    pending_early.append(norm)

                # defer out-projection for this qh
                def outproj(avnT=avnT, wo_sb=wo_sb, t=t, qh=qh):
                    po = psg.tile([128, 1024], f32, tag="G", name=f"po_{t}_{qh}")
                    for mo in range(2):
                        for ki in range(2):
                            nc.tensor.matmul(po[:, mo * 512:(mo + 1) * 512],
                                             wo_sb[ki][:, mo * 128:(mo + 1) * 128],
                                             avnT[ki],
                                             start=(ki == 0), stop=(ki == 1))
                    oT = otp.tile([128, 1024], f32, tag="oT")
                    ev.copy(oT, po, 1024)
                    if DEBUG and t == 0 and qh == 0:
                        nc.sync.dma_start(dbg_f_d[3], oT)
                    for mo in range(2):
                        nc.sync.dma_start(
                            out_d[t, mo * 128:(mo + 1) * 128, qh * 512:qh * 512 + 512],
                            oT[:, mo * 512:(mo + 1) * 512])
                pending_late.append(outproj)

        flush(pending_early)
        flush(pending_late)

    nc.compile()
    _PROGRAM_CACHE["nc"] = nc
    return nc


def _to_bf16(x):
    return np.asarray(x, dtype=np.float32).astype(ml_dtypes.bfloat16)


def _prep_inputs(x, w_qkv, b_qkv, w_out, b_out):
    x2 = np.ascontiguousarray(np.asarray(x, dtype=np.float32)).reshape(2, N_VIEWS, C, S)
    x2 = _to_bf16(x2)
    w_qkv = np.asarray(w_qkv, dtype=np.float32)
    w_out = np.asarray(w_out, dtype=np.float32)
    wT = _to_bf16(np.transpose(w_qkv, (0, 2, 1)))    # [5, 256, 768]
    woT = _to_bf16(np.transpose(w_out, (0, 2, 1)))   # [5, 256, 256]

    in_maps = []
    for core in range(N_CORES):
        slots = _core_slots(core)
        xq = np.zeros((N_SLOTS, C, S), ml_dtypes.bfloat16)
        xkv = np.empty((KV_ROWS, C, S), ml_dtypes.bfloat16)
        w = np.empty((N_SLOTS, C, 3 * C), ml_dtypes.bfloat16)
        wo = np.empty((N_SLOTS, C, C), ml_dtypes.bfloat16)
        for t, (b, i, qh) in enumerate(slots):
            m = MHA_IDX[i]
            if qh is None:
                xq[t] = x2[b, i]
            else:
                xq[t, :, 0:512] = x2[b, i][:, qh * 512:(qh + 1) * 512]
            for j, nb in enumerate(SEL[i]):
                xkv[KVOFF[t] + j] = x2[b, nb]
            w[t] = wT[m]
            wo[t] = woT[m]
        in_maps.append({"xq": xq, "xkv": xkv, "w": w, "wo": wo})
    return in_maps


def _gather_output(results, dtype):
    y = np.empty((2, N_VIEWS, C, S), np.float32)
    for core in range(N_CORES):
        out = results[core]["out"]
        for t, (b, i, qh) in enumerate(_core_slots(core)):
            if qh is None:
                y[b, i] = out[t]
            else:
                y[b, i][:, qh * 512:(qh + 1) * 512] = out[t][:, 0:512]
    return y.reshape(2 * N_VIEWS, C, 32, 32).astype(dtype, copy=False)


def _run(inputs, trace=False, tmpdir=None):
    nc = _build_program()
    in_maps = _prep_inputs(**inputs)
    res = run_bass_kernel_spmd(nc, in_maps, core_ids=list(range(N_CORES)),
                               trace=trace, tmpdir=tmpdir)
    y = _gather_output(res.results, np.asarray(inputs["x"]).dtype)
    return y, res


def kernel(x, w_qkv, b_qkv, w_out, b_out):
    y, _ = _run(dict(x=x, w_qkv=w_qkv, b_qkv=b_qkv, w_out=w_out, b_out=b_out))
    return y
